# revision 1
# baseline (speedup 1.0000x reference)
"""AttentionBlock (GroupNorm + single-head self-attention + residual) on 8 TRN2 cores.

Sharding: data-parallel over batch (B=4) x query-halves (2 per sample) = 8 cores.
Each core gets one full (row-rotated) sample [4096, 512]; the rotation puts that
core's 2048 query rows at rows [0, 2048) so all 8 cores run one identical SPMD
program. Softmax/attention are invariant to key-row permutation, so rotating
keys/values together with the sample is exact.

Per-core pipeline (all layouts chosen so no P/O transposes are ever needed):
  x [4096,512] --PE-transpose--> x^T [512,4096] --stats--> scale/bias per channel
  t^T = normalize(x^T) (bf16)
  K^T = Wk-blocks^T @ t^T   (natural),  Q^T likewise,  V = t^T-blocks^T @ Wv
  S^T[m,q] = K^T-blocks^T @ Q^T        (psum, fp32)
  P^T = exp(scale * S^T)               (no max-subtraction; scores ~ N(0,1))
  denom[q] = ones^T @ P^T              (psum accumulate over m)
  O'^T[c,q] = V-blocks^T @ P^T         (psum accumulate over m)
  proj[q,j] = O'^T-blocks^T @ Wo       (natural [q,j] layout)
  out = proj / denom + bo + x          (residual in fp32)
"""

import math

import numpy as np

import concourse.bacc as bacc
import concourse.mybir as mybir
import concourse.tile as tile
from concourse import bass_utils
from concourse.masks import make_identity

B, HH, WW, C = 4, 64, 64, 512
N = HH * WW          # 4096 tokens per sample
NQ = N // 2          # 2048 queries per core
G = 32               # groupnorm groups
GS = C // G          # 16 channels per group
EPS = 1e-6
SCALE = 1.0 / math.sqrt(C)
N_CORES = 8
F32 = mybir.dt.float32
BF16 = mybir.dt.bfloat16
FP8 = mybir.dt.float8e4
DR_S = True      # DoubleRow fp8 for the S^T matmul
DR_O = True      # DoubleRow fp8 for the O'^T matmul
DR_DEN = True    # DoubleRow fp8 ones-matmul for the softmax denominator
EXP_BIAS = -2.0          # exp(scale*S + bias): keeps fp8 p in [~1e-3, 320]

CT = C // 128        # 4 channel tiles
NT = N // 128        # 32 token tiles
MC = N // 512        # 8 512-wide token chunks
QC = NQ // 512       # 4 query chunks per core


def build_program():
    nc = bacc.Bacc("TRN2", target_bir_lowering=False, debug=False)

    x = nc.dram_tensor("x", [N, C], F32, kind="ExternalInput").ap()
    ws = {
        w: nc.dram_tensor(w, [C, C], F32, kind="ExternalInput").ap()
        for w in ("wq", "wk", "wv", "wo")
    }
    bs = {
        b: nc.dram_tensor(b, [C], F32, kind="ExternalInput").ap()
        for b in ("bq", "bk", "bv", "bo", "gamma", "beta")
    }
    gmap = nc.dram_tensor("gmap", [128, 8], F32, kind="ExternalInput").ap()
    gmapT = nc.dram_tensor("gmapT", [8, 128], F32, kind="ExternalInput").ap()
    out = nc.dram_tensor("out", [NQ, C], F32, kind="ExternalOutput").ap()
    den_dram = nc.dram_tensor("den_scratch", [QC, 512], F32, kind="Internal").ap()

    with tile.TileContext(nc) as tc:
        build_body(tc, x, ws, bs, gmap, gmapT, out, den_dram)
    nc.compile()
    return nc


def build_body(tc, x, ws, bs, gmap, gmapT, out, den_dram):
    nc = tc.nc
    Exp = mybir.ActivationFunctionType.Exp
    Copy = mybir.ActivationFunctionType.Copy
    Ident = mybir.ActivationFunctionType.Identity
    Square = mybir.ActivationFunctionType.Square
    Sqrt = mybir.ActivationFunctionType.Sqrt
    AX = mybir.AxisListType.X

    const = tc.alloc_tile_pool(name="const", bufs=1)
    attn = tc.alloc_tile_pool(name="attn", bufs=1)

    # ---- constants -------------------------------------------------------
    ident = const.tile([128, 128], F32)
    make_identity(nc, ident)
    gmap_sb = const.tile([128, 8], F32)
    nc.sync.dma_start(out=gmap_sb, in_=gmap)
    gmapT_sb = const.tile([8, 128], F32)
    nc.sync.dma_start(out=gmapT_sb, in_=gmapT)
    # per-channel vectors as [128, CT] tiles: [p, i] = vec[i*128 + p]
    chan = {}
    for name in ("bq", "bk", "gamma", "beta"):
        t = const.tile([128, CT], F32, name=f"ch_{name}")
        nc.gpsimd.dma_start(out=t, in_=bs[name].rearrange("(i p) -> p i", p=128))
        chan[name] = t
    def bcast_rows(ap):
        import concourse.bass as bass

        return bass.AP(tensor=ap.tensor, offset=ap.offset, ap=[[0, 128], *ap.ap])

    bv_bc = const.tile([128, C], F32)
    nc.gpsimd.dma_start(out=bv_bc, in_=bcast_rows(bs["bv"]))
    bo_bc = const.tile([128, C], F32)
    nc.gpsimd.dma_start(out=bo_bc, in_=bcast_rows(bs["bo"]))
    ones_mb = const.tile([128, 1], BF16)
    nc.vector.memset(ones_mb, 1.0)
    eps_t = const.tile([8, 1], F32)
    nc.vector.memset(eps_t, EPS)
    scl_t = const.tile([128, 1], F32)
    nc.vector.memset(scl_t, SCALE)
    eb_t = const.tile([128, 1], F32)
    nc.vector.memset(eb_t, EXP_BIAS)
    ones_dr = const.tile([128, 2, 16], FP8)
    nc.vector.memset(ones_dr, 1.0)

    # weights declared up front (bf16), loaded after the x transposes so the
    # x DMA stream owns the queue at kernel start
    w_sb = {
        name: const.tile([128, CT, C], BF16, name=f"{name}_sb")
        for name in ("wq", "wk", "wv", "wo")
    }

    stats = const.tile([128, 8], F32)       # cols 0..3 sum_i, 4..7 sumsq_i
    scale_sb = const.tile([128, CT], F32)
    bias_sb = const.tile([128, CT], F32)

    # persistent attention operands
    kT = attn.tile([128, CT, N], FP8)
    qT = attn.tile([128, CT, NQ], FP8)
    v_sb = attn.tile([128, NT, C], FP8)

    with tc.tile_pool(name="xt_pool", bufs=1) as xt_pool:
        # ---- phase 1: load + transpose + groupnorm stats ----------------
        xT = xt_pool.tile([128, CT, N], F32)    # [p, i, n] = x[n, i*128+p]
        with (
            tc.tile_pool(name="ph1a", bufs=1) as ph1a,
            tc.tile_pool(name="ph1aps", bufs=1, space="PSUM") as ph1aps,
        ):
            sums_blk = ph1a.tile([128, CT, MC], F32)   # per-chunk channel sums
            sq_chunk = ph1a.tile([128, CT, MC], F32)   # per-chunk channel sumsq
            for jg in range(MC):
                stgs = []
                for q in range(4):
                    j = jg * 4 + q
                    stg = ph1a.tile(
                        [128, C], F32, tag=f"xstage{q}", bufs=3, name=f"stg{j}"
                    )
                    nc.sync.dma_start(out=stg, in_=x[j * 128 : (j + 1) * 128, :])
                    stgs.append(stg)
                csl = slice(jg * 512, (jg + 1) * 512)
                for i in range(CT):
                    tp = ph1aps.tile([128, 512], F32, tag="tp", bufs=6, name=f"tp{jg}_{i}")
                    for q in range(4):
                        nc.tensor.transpose(
                            tp[:, q * 128 : (q + 1) * 128],
                            stgs[q][:, i * 128 : (i + 1) * 128],
                            ident,
                        )
                    nc.vector.tensor_scalar(
                        out=xT[:, i, csl], in0=tp, scalar1=0.0, scalar2=0.0,
                        op0=mybir.AluOpType.add, op1=mybir.AluOpType.add,
                        accum_out=sums_blk[:, i, jg : jg + 1],
                    )
                    sqs = ph1a.tile(
                        [128, 512], BF16, tag="sqs", bufs=3, name=f"sq{jg}_{i}"
                    )
                    nc.scalar.activation(
                        out=sqs, in_=xT[:, i, csl], func=Square,
                        accum_out=sq_chunk[:, i, jg : jg + 1],
                    )
            nc.vector.reduce_sum(out=stats[:, 0:4], in_=sums_blk, axis=AX)
            nc.vector.reduce_sum(out=stats[:, 4:8], in_=sq_chunk, axis=AX)

        # weights: casting DMA (SWDGE) straight from DRAM fp32 -> SBUF bf16
        for name in ("wk", "wq", "wv", "wo"):
            for ci in range(CT):
                nc.gpsimd.dma_start(
                    out=w_sb[name][:, ci, :],
                    in_=ws[name][ci * 128 : (ci + 1) * 128, :],
                )

        ph12ps = tc.alloc_tile_pool(name="ph12ps", bufs=1, space="PSUM")
        # group stats: [8, 8] = gmap^T @ stats;  cols 0..3 gsum, 4..7 gsumsq
        gs_ps = ph12ps.tile([8, 8], F32, tag="gs")
        nc.tensor.matmul(gs_ps, lhsT=gmap_sb, rhs=stats, start=True, stop=True)
        gstats = const.tile([8, 8], F32)
        nc.vector.tensor_copy(out=gstats, in_=gs_ps)

        inv_n = 1.0 / (N * GS)
        me_t = const.tile([8, 2 * CT], F32)     # cols 0..3 mean, 4..7 E[x^2]
        nc.vector.tensor_scalar_mul(out=me_t, in0=gstats, scalar1=inv_n)
        var_t = const.tile([8, CT], F32)
        nc.vector.tensor_mul(out=var_t, in0=me_t[:, 0:4], in1=me_t[:, 0:4])
        nc.vector.tensor_sub(out=var_t, in0=me_t[:, 4:8], in1=var_t)
        rstd_t = const.tile([8, CT], F32)
        nc.scalar.activation(out=rstd_t, in_=var_t, func=Sqrt, bias=eps_t)
        nc.vector.reciprocal(out=rstd_t, in_=rstd_t)

        # broadcast per-group -> per-channel: bc_ps cols 2i=mean_i, 2i+1=rstd_i
        bc_ps = ph12ps.tile([128, 2 * CT], F32, tag="bc")
        for i in range(CT):
            nc.tensor.matmul(
                bc_ps[:, 2 * i : 2 * i + 1], lhsT=gmapT_sb,
                rhs=me_t[:, i : i + 1], start=True, stop=True,
            )
            nc.tensor.matmul(
                bc_ps[:, 2 * i + 1 : 2 * i + 2], lhsT=gmapT_sb,
                rhs=rstd_t[:, i : i + 1], start=True, stop=True,
            )
        tmp4 = const.tile([128, CT], F32)
        nc.vector.tensor_mul(
            out=scale_sb, in0=chan["gamma"], in1=bc_ps[:, 1:8:2]
        )
        nc.vector.tensor_mul(out=tmp4, in0=bc_ps[:, 0:8:2], in1=scale_sb)
        nc.vector.tensor_sub(out=bias_sb, in0=chan["beta"], in1=tmp4)

        # ---- phase 2: normalize (chunked) + QKV --------------------------
        tt_pool = tc.alloc_tile_pool(name="tt_pool", bufs=1)
        tT = tt_pool.tile([128, CT, N], BF16)
        for mc in range(MC):
            sl = slice(mc * 512, (mc + 1) * 512)
            for i in range(CT):
                nc.scalar.activation(
                    out=tT[:, i, sl], in_=xT[:, i, sl], func=Ident,
                    bias=bias_sb[:, i : i + 1], scale=scale_sb[:, i : i + 1],
                )
            # K^T chunk
            for i in range(CT):
                ps = ph12ps.tile([128, 512], F32, tag="mm", bufs=4)
                for ci in range(CT):
                    nc.tensor.matmul(
                        ps,
                        lhsT=w_sb["wk"][:, ci, i * 128 : (i + 1) * 128],
                        rhs=tT[:, ci, sl],
                        start=(ci == 0), stop=(ci == CT - 1),
                    )
                nc.vector.tensor_scalar_add(
                    out=kT[:, i, sl], in0=ps, scalar1=chan["bk"][:, i : i + 1]
                )
            # Q^T chunk (tokens [0, NQ) are this core's queries)
            if mc < QC:
                for i in range(CT):
                    ps = ph12ps.tile([128, 512], F32, tag="mm", bufs=4)
                    for ci in range(CT):
                        nc.tensor.matmul(
                            ps,
                            lhsT=w_sb["wq"][:, ci, i * 128 : (i + 1) * 128],
                            rhs=tT[:, ci, sl],
                            start=(ci == 0), stop=(ci == CT - 1),
                        )
                    nc.vector.tensor_scalar_add(
                        out=qT[:, i, sl], in0=ps, scalar1=chan["bq"][:, i : i + 1]
                    )
            # V m-tiles of this chunk
            for ml in range(4):
                m = mc * 4 + ml
                ps = ph12ps.tile([128, 512], F32, tag="mm", bufs=4)
                for ci in range(CT):
                    nc.tensor.matmul(
                        ps,
                        lhsT=tT[:, ci, m * 128 : (m + 1) * 128],
                        rhs=w_sb["wv"][:, ci, :],
                        start=(ci == 0), stop=(ci == CT - 1),
                    )
                nc.vector.tensor_add(out=v_sb[:, m, :], in0=ps, in1=bv_bc)

        tt_pool.release()
        ph12ps.release()

    # ---- phase 3: attention ---------------------------------------------
    with (
        tc.tile_pool(name="ph3", bufs=1) as ph3,
        tc.tile_pool(name="ph3ps", bufs=1, space="PSUM") as ph3ps,
    ):
        def finish_qc(qc, oT, rd):
            """proj + final residual for a finished q-chunk (emitted deferred)."""
            for s in range(4):
                pr_ps = ph3ps.tile([128, 512], F32, tag="pr", name=f"pr{qc}_{s}")
                for ci in range(CT):
                    nc.tensor.matmul(
                        pr_ps,
                        lhsT=oT[:, ci, s * 128 : (s + 1) * 128],
                        rhs=w_sb["wo"][:, ci, :],
                        start=(ci == 0), stop=(ci == CT - 1),
                    )
                row0 = qc * 512 + s * 128
                xr = ph3.tile([128, C], F32, tag="xr", bufs=3, name=f"xr{qc}_{s}")
                nc.sync.dma_start(out=xr, in_=x[row0 : row0 + 128, :])
                fin = ph3.tile([128, C], F32, tag="fin", bufs=3, name=f"fin{qc}_{s}")
                nc.vector.tensor_scalar_mul(
                    out=fin, in0=pr_ps, scalar1=rd[:, s : s + 1]
                )
                nc.vector.tensor_add(out=fin, in0=fin, in1=bo_bc)
                nc.vector.tensor_add(out=fin, in0=fin, in1=xr)
                nc.sync.dma_start(out=out[row0 : row0 + 128, :], in_=fin)

        deferred = None
        for qc in range(QC):
            qsl = slice(qc * 512, (qc + 1) * 512)
            o_ps = [ph3ps.tile([128, 512], F32, tag=f"o{i}", name=f"ops{qc}_{i}") for i in range(CT)]
            den_ps = ph3ps.tile([1, 512], F32, tag="den", bufs=1, name=f"dps{qc}")
            prev = None
            p_pair = None
            for m in range(NT):
                s_ps = ph3ps.tile([128, 512], F32, tag="s", bufs=2, name=f"sps{qc}_{m}")
                if DR_S:
                    for a in range(2):
                        nc.tensor.matmul(
                            s_ps,
                            lhsT=kT[:, 2 * a : 2 * a + 2, m * 128 : (m + 1) * 128],
                            rhs=qT[:, 2 * a : 2 * a + 2, qsl],
                            start=(a == 0), stop=(a == 1),
                            perf_mode=mybir.MatmulPerfMode.DoubleRow,
                        )
                else:
                    for ci in range(CT):
                        nc.tensor.matmul(
                            s_ps,
                            lhsT=kT[:, ci, m * 128 : (m + 1) * 128],
                            rhs=qT[:, ci, qsl],
                            start=(ci == 0), stop=(ci == CT - 1),
                        )
                if m % 2 == 0:
                    p_pair = ph3.tile(
                        [128, 2, 512], FP8, tag="p", bufs=5, name=f"pt{qc}_{m}"
                    )
                nc.scalar.activation(
                    out=p_pair[:, m % 2, :], in_=s_ps, func=Exp,
                    scale=scl_t, bias=eb_t,
                )
                if m % 2 == 1:
                    if prev is not None:
                        emit_pv(nc, den_ps, o_ps, v_sb, ones_dr, *prev)
                    prev = (p_pair, m // 2)
                if m == 2 and deferred is not None:
                    finish_qc(*deferred)
                    deferred = None
            emit_pv(nc, den_ps, o_ps, v_sb, ones_dr, *prev)

            # denominator -> per-q-subtile reciprocal
            den_sb = ph3.tile([1, 512], F32, tag="den_sb", bufs=2, name=f"dsb{qc}")
            nc.vector.tensor_copy(out=den_sb, in_=den_ps)
            nc.sync.dma_start(out=den_dram[qc : qc + 1, :], in_=den_sb)
            rd = ph3.tile([128, 4], F32, tag="rd", bufs=2, name=f"rd{qc}")
            nc.gpsimd.dma_start(
                out=rd, in_=den_dram[qc, :].rearrange("(s p) -> p s", p=128)
            )
            nc.vector.reciprocal(out=rd, in_=rd)

            oT = ph3.tile([128, CT, 512], BF16, tag="oT", bufs=2, name=f"oT{qc}")
            for i in range(CT):
                nc.vector.tensor_copy(out=oT[:, i, :], in_=o_ps[i])
            deferred = (qc, oT, rd)
        finish_qc(*deferred)

    attn.release()
    const.release()


def emit_pv(nc, den_ps, o_ps, v_sb, ones_dr, p_pair, b):
    if DR_DEN:
        nc.tensor.matmul(
            den_ps, lhsT=ones_dr[:, :, 0:1], rhs=p_pair,
            start=(b == 0), stop=(b == NT // 2 - 1), skip_group_check=True,
            perf_mode=mybir.MatmulPerfMode.DoubleRow,
        )
    else:
        for h in range(2):
            nc.tensor.matmul(
                den_ps, lhsT=ones_dr[:, 0, 0:1], rhs=p_pair[:, h, :],
                start=(b == 0 and h == 0), stop=(b == NT // 2 - 1 and h == 1),
                skip_group_check=True,
            )
    for i in range(CT):
        if DR_O:
            nc.tensor.matmul(
                o_ps[i],
                lhsT=v_sb[:, 2 * b : 2 * b + 2, i * 128 : (i + 1) * 128],
                rhs=p_pair,
                start=(b == 0), stop=(b == NT // 2 - 1), skip_group_check=True,
                perf_mode=mybir.MatmulPerfMode.DoubleRow,
            )
        else:
            for h in range(2):
                nc.tensor.matmul(
                    o_ps[i],
                    lhsT=v_sb[:, 2 * b + h, i * 128 : (i + 1) * 128],
                    rhs=p_pair[:, h, :],
                    start=(b == 0 and h == 0),
                    stop=(b == NT // 2 - 1 and h == 1),
                    skip_group_check=True,
                )


_prog_cache = None


def get_program():
    global _prog_cache
    if _prog_cache is None:
        _prog_cache = build_program()
    return _prog_cache


def make_gmaps():
    gmap = np.zeros((128, 8), np.float32)
    gmap[np.arange(128), np.arange(128) // GS] = 1.0
    return gmap, np.ascontiguousarray(gmap.T)


def make_in_maps(inputs):
    x = np.asarray(inputs["x"], np.float32)          # [B, H, W, C]
    gmap, gmapT = make_gmaps()
    common = {
        "wq": np.ascontiguousarray(np.asarray(inputs["Wq"], np.float32)),
        "wk": np.ascontiguousarray(np.asarray(inputs["Wk"], np.float32)),
        "wv": np.ascontiguousarray(np.asarray(inputs["Wv"], np.float32)),
        "wo": np.ascontiguousarray(np.asarray(inputs["Wo"], np.float32)),
        "bq": np.ascontiguousarray(np.asarray(inputs["bq"], np.float32)),
        "bk": np.ascontiguousarray(np.asarray(inputs["bk"], np.float32)),
        "bv": np.ascontiguousarray(np.asarray(inputs["bv"], np.float32)),
        "bo": np.ascontiguousarray(np.asarray(inputs["bo"], np.float32)),
        "gamma": np.ascontiguousarray(np.asarray(inputs["gn_gamma"], np.float32)),
        "beta": np.ascontiguousarray(np.asarray(inputs["gn_beta"], np.float32)),
        "gmap": gmap,
        "gmapT": gmapT,
    }
    in_maps = []
    for core in range(N_CORES):
        b, h = divmod(core, 2)
        xs = x[b].reshape(N, C)
        if h:
            xs = np.roll(xs, -NQ, axis=0)
        in_maps.append({"x": np.ascontiguousarray(xs), **common})
    return in_maps


def assemble(results):
    full = np.empty((B, N, C), np.float32)
    for core in range(N_CORES):
        b, h = divmod(core, 2)
        full[b, h * NQ : (h + 1) * NQ] = results[core]["out"]
    return full.reshape(B, HH, WW, C)


def kernel(**inputs) -> np.ndarray:
    in_maps = make_in_maps(inputs)
    nc = get_program()
    res = bass_utils.run_bass_kernel_spmd(nc, in_maps, core_ids=list(range(N_CORES)))
    return assemble(res.results)



# revision 22
# speedup vs baseline: 1.2041x; 1.2041x over previous
"""AttentionBlock (GroupNorm + single-head self-attention + residual) on 8 TRN2 cores.

Sharding: data-parallel over batch (B=4) x query-halves (2 per sample) = 8 cores.
Each core gets one full (row-rotated) sample [4096, 512]; the rotation puts that
core's 2048 query rows at rows [0, 2048) so all 8 cores run one identical SPMD
program. Softmax/attention are invariant to key-row permutation, so rotating
keys/values together with the sample is exact.

Host prep: x cast to bf16; weights cast to fp8 scaled by 8 (avoids fp8
subnormals); Wo folded into Wv (Wvo = Wv @ Wo) which eliminates the output
projection matmul; bv folded into bo2 = bo + bv @ Wo.

Per-core pipeline (fp8 DoubleRow for every large matmul):
  ph1: x [4096,512] bf16 --PE-transpose--> xT bf16; Act evacs PSUM->SBUF with
       channel-sum accum; DVE squares with accum -> groupnorm stats.
  ph2: tT = fp8(scale*xT + bias) (DVE);  K^T/Q^T = W8-blocks^T @ tT (DR fp8,
       Act/DVE evac + 8*bias);  v2 = tT-blocks^T @ W8vo (DR fp8, Pool evac).
  ph3: S^T[m,q] pairs (DR fp8) -> exp (Act, merged [128,1024]) -> P (fp8 SBUF);
       den[q] = ones^T P (DR);  O'^T[c,q] = v2^T P (DR, deferred one q-chunk);
       PE-transpose O' -> [q,c];  out = O'*(1/(8 den)) + resid + bo2 (Pool).
"""

import math

import numpy as np
import ml_dtypes

import concourse.bacc as bacc
import concourse.mybir as mybir
import concourse.tile as tile
from concourse import bass_utils
from concourse.masks import make_identity

B, HH, WW, C = 4, 64, 64, 512
N = HH * WW          # 4096 tokens per sample
NQ = N // 2          # 2048 queries per core
G = 32               # groupnorm groups
GS = C // G          # 16 channels per group
EPS = 1e-6
SCALE = 1.0 / math.sqrt(C)
N_CORES = 8
F32 = mybir.dt.float32
BF16 = mybir.dt.bfloat16
FP8 = mybir.dt.float8e4
W_SCALE = 8.0            # weights stored as fp8(8*W)
EXP_BIAS = -2.0          # exp(scale*S + bias): keeps fp8 p in [~1e-3, 320]

CT = C // 128        # 4 channel tiles
NT = N // 128        # 32 token tiles
JG = N // 1024       # 4 1024-token groups (phase 1)
CP = N // 1024       # 4 1024-token chunk-pairs (phase 2)
QC = NQ // 512       # 4 query chunks per core
NPAIR = NT // 2      # 16 m-tile pairs per q-chunk


def build_program():
    nc = bacc.Bacc("TRN2", target_bir_lowering=False, debug=False)

    x = nc.dram_tensor("x", [N, C], BF16, kind="ExternalInput").ap()
    ws = {
        w: nc.dram_tensor(w, [C, C], FP8, kind="ExternalInput").ap()
        for w in ("wq", "wk", "wvo")
    }
    bs = {
        b: nc.dram_tensor(b, [C], F32, kind="ExternalInput").ap()
        for b in ("bq8", "bk8", "bo2", "gamma", "beta")
    }
    gmap = nc.dram_tensor("gmap", [128, 8], F32, kind="ExternalInput").ap()
    gmapT = nc.dram_tensor("gmapT", [8, 128], F32, kind="ExternalInput").ap()
    out = nc.dram_tensor("out", [NQ, C], F32, kind="ExternalOutput").ap()
    den_dram = nc.dram_tensor("den_scratch", [QC, 512], F32, kind="Internal").ap()

    with tile.TileContext(nc) as tc:
        build_body(tc, x, ws, bs, gmap, gmapT, out, den_dram)
    nc.compile()
    return nc


def build_body(tc, x, ws, bs, gmap, gmapT, out, den_dram):
    nc = tc.nc
    Exp = mybir.ActivationFunctionType.Exp
    Copy = mybir.ActivationFunctionType.Copy
    Ident = mybir.ActivationFunctionType.Identity
    Sqrt = mybir.ActivationFunctionType.Sqrt
    AX = mybir.AxisListType.X
    DR = mybir.MatmulPerfMode.DoubleRow
    MUL = mybir.AluOpType.mult
    ADD = mybir.AluOpType.add

    const = tc.alloc_tile_pool(name="const", bufs=1)
    attn = tc.alloc_tile_pool(name="attn", bufs=1)
    resid_pool = tc.alloc_tile_pool(name="resid", bufs=1)

    # ---- constants -------------------------------------------------------
    ident_bf = const.tile([128, 128], BF16)
    make_identity(nc, ident_bf)
    gmap_sb = const.tile([128, 8], F32)
    nc.sync.dma_start(out=gmap_sb, in_=gmap)
    gmapT_sb = const.tile([8, 128], F32)
    nc.sync.dma_start(out=gmapT_sb, in_=gmapT)
    # per-channel vectors as [128, CT] tiles: [p, i] = vec[i*128 + p]
    chan = {}
    for name in ("bq8", "bk8", "gamma", "beta"):
        t = const.tile([128, CT], F32, name=f"ch_{name}")
        nc.gpsimd.dma_start(out=t, in_=bs[name].rearrange("(i p) -> p i", p=128))
        chan[name] = t

    def bcast_rows(ap):
        import concourse.bass as bass

        return bass.AP(tensor=ap.tensor, offset=ap.offset, ap=[[0, 128], *ap.ap])

    bo2_bc = const.tile([128, C], F32)
    nc.gpsimd.dma_start(out=bo2_bc, in_=bcast_rows(bs["bo2"]))
    eps_t = const.tile([8, 1], F32)
    nc.vector.memset(eps_t, EPS)
    scl_t = const.tile([128, 1], F32)
    nc.vector.memset(scl_t, SCALE / (W_SCALE * W_SCALE))
    eb_t = const.tile([128, 1], F32)
    nc.vector.memset(eb_t, EXP_BIAS)
    ones_dr = const.tile([128, 2, 16], FP8)
    nc.vector.memset(ones_dr, 1.0)

    # weights (fp8, pre-scaled x8 on host); loaded after the x stream starts
    w_sb = {
        name: const.tile([128, CT, C], FP8, name=f"{name}_sb")
        for name in ("wq", "wk", "wvo")
    }

    sums_blk = const.tile([128, CT, JG], F32)
    sq_blk = const.tile([128, CT, JG], F32)
    stats = const.tile([128, 8], F32)       # cols 0..3 sum_i, 4..7 sumsq_i
    scale_sb = const.tile([128, CT], F32)
    bias_sb = const.tile([128, CT], F32)

    # persistent attention operands (fp8)
    kT = attn.tile([128, CT, N], FP8)
    qT = attn.tile([128, CT, NQ], FP8)
    v_sb = attn.tile([128, NT, C], FP8)

    # residual rows (this core's 2048 query rows), bf16, kept to the end
    resid = resid_pool.tile([128, 16, 512], BF16)

    xt_pool = tc.alloc_tile_pool(name="xt_pool", bufs=1)
    xT = xt_pool.tile([128, CT, N], BF16)    # [p, i, n] = x[n, i*128+p]

    # ---- phase 1: load + transpose + groupnorm stats ---------------------
    with (
        tc.tile_pool(name="ph1a", bufs=1) as ph1a,
        tc.tile_pool(name="ph1ps", bufs=1, space="PSUM") as ph1ps,
    ):
        for jg in range(JG):
            stgs = []
            for q in range(8):
                j = jg * 8 + q
                if j < 16:
                    stg = resid[:, j, :]
                    nc.sync.dma_start(out=stg, in_=x[j * 128 : (j + 1) * 128, :])
                else:
                    stg = ph1a.tile(
                        [128, 512], BF16, tag=f"xstage{q % 4}", bufs=2, name=f"stg{j}"
                    )
                    nc.sync.dma_start(out=stg, in_=x[j * 128 : (j + 1) * 128, :])
                stgs.append(stg)
            gsl = slice(jg * 1024, (jg + 1) * 1024)
            for i in range(CT):
                tp = ph1ps.tile([128, 8, 128], BF16, tag="tp", bufs=4, name=f"tp{jg}_{i}")
                for q in range(8):
                    nc.tensor.transpose(
                        tp[:, q, :],
                        stgs[q][:, i * 128 : (i + 1) * 128],
                        ident_bf,
                    )
                # PSUM -> SBUF bf16 evac, with per-channel sum accumulation
                nc.scalar.activation(
                    out=xT[:, i, gsl], in_=tp, func=Copy,
                    accum_out=sums_blk[:, i, jg : jg + 1],
                )
                sqs = ph1a.tile(
                    [128, 1024], BF16, tag="sqs", bufs=2, name=f"sq{jg}_{i}"
                )
                nc.vector.tensor_mul(out=sqs, in0=xT[:, i, gsl], in1=xT[:, i, gsl])
                nc.vector.tensor_scalar(
                    out=sqs, in0=sqs, scalar1=0.0, scalar2=0.0,
                    op0=ADD, op1=ADD,
                    accum_out=sq_blk[:, i, jg : jg + 1],
                )

        # weights can stream in behind the x stages
        for name in ("wk", "wq", "wvo"):
            for ci in range(CT):
                nc.sync.dma_start(
                    out=w_sb[name][:, ci, :],
                    in_=ws[name][ci * 128 : (ci + 1) * 128, :],
                )

        nc.vector.reduce_sum(out=stats[:, 0:4], in_=sums_blk, axis=AX)
        nc.vector.reduce_sum(out=stats[:, 4:8], in_=sq_blk, axis=AX)

    ph23ps = tc.alloc_tile_pool(name="ph23ps", bufs=1, space="PSUM")
    # group stats: [8, 8] = gmap^T @ stats;  cols 0..3 gsum, 4..7 gsumsq
    gs_ps = ph23ps.tile([8, 8], F32, tag="gs", bufs=1)
    nc.tensor.matmul(gs_ps, lhsT=gmap_sb, rhs=stats, start=True, stop=True)
    gstats = const.tile([8, 8], F32)
    nc.vector.tensor_copy(out=gstats, in_=gs_ps)

    inv_n = 1.0 / (N * GS)
    me_t = const.tile([8, 2 * CT], F32)     # cols 0..3 mean, 4..7 E[x^2]
    nc.vector.tensor_scalar_mul(out=me_t, in0=gstats, scalar1=inv_n)
    var_t = const.tile([8, CT], F32)
    nc.vector.tensor_mul(out=var_t, in0=me_t[:, 0:4], in1=me_t[:, 0:4])
    nc.vector.tensor_sub(out=var_t, in0=me_t[:, 4:8], in1=var_t)
    rstd_t = const.tile([8, CT], F32)
    nc.scalar.activation(out=rstd_t, in_=var_t, func=Sqrt, bias=eps_t)
    nc.vector.reciprocal(out=rstd_t, in_=rstd_t)

    # broadcast per-group -> per-channel: bc_ps cols 2i=mean_i, 2i+1=rstd_i
    bc_ps = ph23ps.tile([128, 2 * CT], F32, tag="gs", bufs=1, name="bc")
    for i in range(CT):
        nc.tensor.matmul(
            bc_ps[:, 2 * i : 2 * i + 1], lhsT=gmapT_sb,
            rhs=me_t[:, i : i + 1], start=True, stop=True,
        )
        nc.tensor.matmul(
            bc_ps[:, 2 * i + 1 : 2 * i + 2], lhsT=gmapT_sb,
            rhs=rstd_t[:, i : i + 1], start=True, stop=True,
        )
    tmp4 = const.tile([128, CT], F32)
    nc.vector.tensor_mul(out=scale_sb, in0=chan["gamma"], in1=bc_ps[:, 1:8:2])
    nc.vector.tensor_mul(out=tmp4, in0=bc_ps[:, 0:8:2], in1=scale_sb)
    nc.vector.tensor_sub(out=bias_sb, in0=chan["beta"], in1=tmp4)

    # ---- phase 2: normalize (fp8) + K/Q/V --------------------------------
    # PSUM evacuations are legal only on Act/DVE; alternate between them.
    # PSUM_SPLIT: emit PSUM reads per 2KB bank (in case HW can't cross banks).
    PSUM_SPLIT = True
    evac_rr = [0]

    def psum_evac(out, in_, bias=None):
        if PSUM_SPLIT:
            # fp32 [128, 2, 512] psum tile = 2 banks; split reads per bank
            if len(out.shape) == 3:
                pieces = [(out[:, 0, :], in_[:, 0, :]), (out[:, 1, :], in_[:, 1, :])]
            else:
                pieces = [(out[:, 0:512], in_[:, 0, :]), (out[:, 512:1024], in_[:, 1, :])]
        else:
            pieces = [(out, in_)]
        evac_rr[0] ^= 1
        for o_, i_ in pieces:
            if evac_rr[0]:
                nc.scalar.activation(
                    out=o_, in_=i_, func=(Ident if bias is not None else Copy),
                    **({"bias": bias} if bias is not None else {}),
                )
            elif bias is not None:
                nc.vector.tensor_scalar_add(out=o_, in0=i_, scalar1=bias)
            else:
                nc.vector.tensor_copy(out=o_, in_=i_)

    tt_pool = tc.alloc_tile_pool(name="tt_pool", bufs=1)
    tT = tt_pool.tile([128, CT, N], FP8)
    for cp in range(CP):
        sl = slice(cp * 1024, (cp + 1) * 1024)
        for i in range(CT):
            # normalize (SBUF->SBUF)
            GPSIMD_COMPUTE = False
            eng = nc.gpsimd if GPSIMD_COMPUTE else nc.vector
            eng.tensor_scalar(
                out=tT[:, i, sl], in0=xT[:, i, sl],
                scalar1=scale_sb[:, i : i + 1], scalar2=bias_sb[:, i : i + 1],
                op0=MUL, op1=ADD,
            )
        # K^T chunk-pair: DR fp8, Act evac (+8*bk)
        for i in range(CT):
            kps = ph23ps.tile([128, 2, 512], F32, tag="mm", bufs=3, name=f"k{cp}_{i}")
            for h in range(2):
                hsl = slice(cp * 1024 + h * 512, cp * 1024 + (h + 1) * 512)
                for a in range(2):
                    nc.tensor.matmul(
                        kps[:, h, :],
                        lhsT=w_sb["wk"][:, 2 * a : 2 * a + 2, i * 128 : (i + 1) * 128],
                        rhs=tT[:, 2 * a : 2 * a + 2, hsl],
                        start=(a == 0), stop=(a == 1),
                        perf_mode=DR,
                    )
            psum_evac(kT[:, i, sl], kps, bias=chan["bk8"][:, i : i + 1])
        # Q^T chunk-pair (tokens [0, NQ) are this core's queries): DVE evac
        if cp < NQ // 1024:
            for i in range(CT):
                qps = ph23ps.tile([128, 2, 512], F32, tag="mm", bufs=3, name=f"q{cp}_{i}")
                for h in range(2):
                    hsl = slice(cp * 1024 + h * 512, cp * 1024 + (h + 1) * 512)
                    for a in range(2):
                        nc.tensor.matmul(
                            qps[:, h, :],
                            lhsT=w_sb["wq"][:, 2 * a : 2 * a + 2, i * 128 : (i + 1) * 128],
                            rhs=tT[:, 2 * a : 2 * a + 2, hsl],
                            start=(a == 0), stop=(a == 1),
                            perf_mode=DR,
                        )
                psum_evac(qT[:, i, sl], qps, bias=chan["bq8"][:, i : i + 1])
        # v2 m-tiles of this chunk-pair (Wvo fused; no bias): Pool evac
        for mp in range(4):
            m0 = cp * 8 + 2 * mp
            vps = ph23ps.tile([128, 2, 512], F32, tag="mm", bufs=3, name=f"v{cp}_{mp}")
            for h in range(2):
                m = m0 + h
                for a in range(2):
                    nc.tensor.matmul(
                        vps[:, h, :],
                        lhsT=tT[:, 2 * a : 2 * a + 2, m * 128 : (m + 1) * 128],
                        rhs=w_sb["wvo"][:, 2 * a : 2 * a + 2, :],
                        start=(a == 0), stop=(a == 1),
                        perf_mode=DR,
                    )
            psum_evac(v_sb[:, m0 : m0 + 2, :], vps)

    tt_pool.release()
    ph23ps.release()
    xt_pool.release()

    # ---- phase 3: attention ---------------------------------------------
    with (
        tc.tile_pool(name="ph3", bufs=1) as ph3,
        tc.tile_pool(name="ph3ps", bufs=1, space="PSUM") as ph3ps,
    ):
        def emit_O_half(st, ho):
            """O'^T channel half [2*ho*128, (2*ho+2)*128) for a finished q-chunk."""
            qc, p_all, oT, _rd = st
            o_ps = ph3ps.tile(
                [128, 2, 512], F32, tag="o", bufs=1, name=f"o{qc}_{ho}"
            )
            for b in range(NPAIR):
                for i2 in range(2):
                    i = 2 * ho + i2
                    nc.tensor.matmul(
                        o_ps[:, i2, :],
                        lhsT=v_sb[:, 2 * b : 2 * b + 2, i * 128 : (i + 1) * 128],
                        rhs=p_all[:, 2 * b : 2 * b + 2, :],
                        start=(b == 0), stop=(b == NPAIR - 1),
                        skip_group_check=True,
                        perf_mode=DR,
                    )
            nc.vector.tensor_copy(out=oT[:, 2 * ho : 2 * ho + 2, :], in_=o_ps)

        def emit_fin(st, s):
            """transpose + scale + residual + store for one 128-row out tile."""
            qc, _p_all, oT, rd = st
            ftr = ph3ps.tile([128, 512], BF16, tag="ftr", bufs=1, name=f"ftr{qc}_{s}")
            for i in range(CT):
                nc.tensor.transpose(
                    ftr[:, i * 128 : (i + 1) * 128],
                    oT[:, i, s * 128 : (s + 1) * 128],
                    ident_bf,
                )
            fin = ph3.tile([128, C], F32, tag="fin", bufs=3, name=f"fin{qc}_{s}")
            nc.vector.scalar_tensor_tensor(
                out=fin, in0=ftr, scalar=rd[:, s : s + 1],
                in1=resid[:, qc * 4 + s, :], op0=MUL, op1=ADD,
            )
            nc.vector.tensor_add(out=fin, in0=fin, in1=bo2_bc)
            row0 = qc * 512 + s * 128
            nc.sync.dma_start(out=out[row0 : row0 + 128, :], in_=fin)

        prev = None       # (qc, p_all, oT, rd) of the previous q-chunk
        for qc in range(QC):
            qsl = slice(qc * 512, (qc + 1) * 512)
            p_all = ph3.tile([128, NT, 512], FP8, tag="p", bufs=2, name=f"p{qc}")
            oT = ph3.tile([128, CT, 512], BF16, tag="oT", bufs=2, name=f"oT{qc}")
            den_ps = ph3ps.tile([1, 512], F32, tag="den", bufs=1, name=f"dps{qc}")
            for b in range(NPAIR):
                s_big = ph3ps.tile(
                    [128, 2, 512], F32, tag="s", bufs=2, name=f"s{qc}_{b}"
                )
                for h in range(2):
                    m = 2 * b + h
                    for a in range(2):
                        nc.tensor.matmul(
                            s_big[:, h, :],
                            lhsT=kT[:, 2 * a : 2 * a + 2, m * 128 : (m + 1) * 128],
                            rhs=qT[:, 2 * a : 2 * a + 2, qsl],
                            start=(a == 0), stop=(a == 1),
                            perf_mode=DR,
                        )
                for h in range(2):
                    nc.scalar.activation(
                        out=p_all[:, 2 * b + h, :], in_=s_big[:, h, :], func=Exp,
                        scale=scl_t, bias=eb_t,
                    )
                nc.tensor.matmul(
                    den_ps, lhsT=ones_dr[:, :, 0:1],
                    rhs=p_all[:, 2 * b : 2 * b + 2, :],
                    start=(b == 0), stop=(b == NPAIR - 1),
                    skip_group_check=True,
                    perf_mode=DR,
                )
                if prev is not None:
                    if b == 2:
                        emit_O_half(prev, 0)
                    elif b == 6:
                        emit_O_half(prev, 1)
                    elif b == 10:
                        emit_fin(prev, 0)
                        emit_fin(prev, 1)
                    elif b == 13:
                        emit_fin(prev, 2)
                        emit_fin(prev, 3)

            # denominator (x8 for the fp8 weight scale): DMA-transpose to rd
            den_sb = ph3.tile([1, 512], F32, tag="den_sb", bufs=2, name=f"dsb{qc}")
            nc.vector.tensor_scalar_mul(out=den_sb, in0=den_ps, scalar1=W_SCALE)
            nc.sync.dma_start(out=den_dram[qc : qc + 1, :], in_=den_sb)
            rd = ph3.tile([128, 4], F32, tag="rd", bufs=2, name=f"rd{qc}")
            nc.gpsimd.dma_start(
                out=rd, in_=den_dram[qc, :].rearrange("(s p) -> p s", p=128)
            )
            nc.vector.reciprocal(out=rd, in_=rd)
            prev = (qc, p_all, oT, rd)

        emit_O_half(prev, 0)
        emit_O_half(prev, 1)
        for s in range(4):
            emit_fin(prev, s)

    resid_pool.release()
    attn.release()
    const.release()


_prog_cache = None


def get_program():
    global _prog_cache
    if _prog_cache is None:
        _prog_cache = build_program()
    return _prog_cache


def make_gmaps():
    gmap = np.zeros((128, 8), np.float32)
    gmap[np.arange(128), np.arange(128) // GS] = 1.0
    return gmap, np.ascontiguousarray(gmap.T)


def make_in_maps(inputs):
    x = np.asarray(inputs["x"], np.float32)          # [B, H, W, C]
    gmap, gmapT = make_gmaps()
    f32 = np.float32
    Wq = np.asarray(inputs["Wq"], f32)
    Wk = np.asarray(inputs["Wk"], f32)
    Wv = np.asarray(inputs["Wv"], f32)
    Wo = np.asarray(inputs["Wo"], f32)
    Wvo = (Wv @ Wo).astype(f32)
    bo2 = (np.asarray(inputs["bo"], f32)
           + np.asarray(inputs["bv"], f32) @ Wo).astype(f32)

    def fp8(a):
        return np.ascontiguousarray(np.asarray(a, dtype=ml_dtypes.float8_e4m3))

    common = {
        "wq": fp8(W_SCALE * Wq),
        "wk": fp8(W_SCALE * Wk),
        "wvo": fp8(W_SCALE * Wvo),
        "bq8": np.ascontiguousarray(W_SCALE * np.asarray(inputs["bq"], f32)),
        "bk8": np.ascontiguousarray(W_SCALE * np.asarray(inputs["bk"], f32)),
        "bo2": np.ascontiguousarray(bo2),
        "gamma": np.ascontiguousarray(np.asarray(inputs["gn_gamma"], f32)),
        "beta": np.ascontiguousarray(np.asarray(inputs["gn_beta"], f32)),
        "gmap": gmap,
        "gmapT": gmapT,
    }
    in_maps = []
    for core in range(N_CORES):
        b, h = divmod(core, 2)
        xs = x[b].reshape(N, C)
        if h:
            xs = np.roll(xs, -NQ, axis=0)
        in_maps.append(
            {"x": np.ascontiguousarray(xs.astype(ml_dtypes.bfloat16)), **common}
        )
    return in_maps


def assemble(results):
    full = np.empty((B, N, C), np.float32)
    for core in range(N_CORES):
        b, h = divmod(core, 2)
        full[b, h * NQ : (h + 1) * NQ] = results[core]["out"]
    return full.reshape(B, HH, WW, C)


def kernel(**inputs) -> np.ndarray:
    in_maps = make_in_maps(inputs)
    nc = get_program()
    res = bass_utils.run_bass_kernel_spmd(nc, in_maps, core_ids=list(range(N_CORES)))
    return assemble(res.results)


# revision 26
# speedup vs baseline: 1.3766x; 1.1432x over previous
"""AttentionBlock (GroupNorm + single-head self-attention + residual) on 8 TRN2 cores.

Sharding: data-parallel over batch (B=4) x query-halves (2 per sample) = 8 cores.
Each core gets one full (row-rotated) sample [4096, 512]; the rotation puts that
core's 2048 query rows at rows [0, 2048) so all 8 cores run one identical SPMD
program. Softmax/attention are invariant to key-row permutation, so rotating
keys/values together with the sample is exact.

Host prep: x cast to bf16; weights cast to fp8 scaled by 8 (avoids fp8
subnormals); Wo folded into Wv (Wvo = Wv @ Wo) which eliminates the output
projection matmul; bv folded into bo2 = bo + bv @ Wo.

Per-core pipeline (fp8 DoubleRow for every large matmul):
  ph1: x [4096,512] bf16 --PE-transpose--> xT bf16; Act evacs PSUM->SBUF with
       channel-sum accum; DVE squares with accum -> groupnorm stats.
  ph2: tT = fp8(scale*xT + bias) (DVE);  K^T/Q^T = W8-blocks^T @ tT (DR fp8,
       Act/DVE evac + 8*bias);  v2 = tT-blocks^T @ W8vo (DR fp8, Pool evac).
  ph3: S^T[m,q] pairs (DR fp8) -> exp (Act, merged [128,1024]) -> P (fp8 SBUF);
       den[q] = ones^T P (DR);  O'^T[c,q] = v2^T P (DR, deferred one q-chunk);
       PE-transpose O' -> [q,c];  out = O'*(1/(8 den)) + resid + bo2 (Pool).
"""

import math

import numpy as np
import ml_dtypes

import concourse.bacc as bacc
import concourse.mybir as mybir
import concourse.tile as tile
from concourse import bass_utils
from concourse.masks import make_identity

B, HH, WW, C = 4, 64, 64, 512
N = HH * WW          # 4096 tokens per sample
NQ = N // 2          # 2048 queries per core
G = 32               # groupnorm groups
GS = C // G          # 16 channels per group
EPS = 1e-6
SCALE = 1.0 / math.sqrt(C)
N_CORES = 8
F32 = mybir.dt.float32
BF16 = mybir.dt.bfloat16
FP8 = mybir.dt.float8e4
W_SCALE = 8.0            # weights stored as fp8(8*W)
EXP_BIAS = -2.0          # exp(scale*S + bias): keeps fp8 p in [~1e-3, 320]

CT = C // 128        # 4 channel tiles
NT = N // 128        # 32 token tiles
JG = N // 1024       # 4 1024-token groups (phase 1)
CP = N // 1024       # 4 1024-token chunk-pairs (phase 2)
QC = NQ // 512       # 4 query chunks per core
NPAIR = NT // 2      # 16 m-tile pairs per q-chunk


def build_program():
    nc = bacc.Bacc("TRN2", target_bir_lowering=False, debug=False)

    x = nc.dram_tensor("x", [N, C], BF16, kind="ExternalInput").ap()
    ws = {
        w: nc.dram_tensor(w, [C, C], FP8, kind="ExternalInput").ap()
        for w in ("wq", "wk", "wvo")
    }
    bs = {
        b: nc.dram_tensor(b, [C], F32, kind="ExternalInput").ap()
        for b in ("bq8", "bk8", "bo2", "gamma", "beta")
    }
    gmap = nc.dram_tensor("gmap", [128, 8], F32, kind="ExternalInput").ap()
    gmapT = nc.dram_tensor("gmapT", [8, 128], F32, kind="ExternalInput").ap()
    out = nc.dram_tensor("out", [NQ, C], F32, kind="ExternalOutput").ap()
    den_dram = nc.dram_tensor("den_scratch", [QC, 512], F32, kind="Internal").ap()

    with tile.TileContext(nc) as tc:
        build_body(tc, x, ws, bs, gmap, gmapT, out, den_dram)
    nc.compile()
    return nc


def build_body(tc, x, ws, bs, gmap, gmapT, out, den_dram):
    nc = tc.nc
    Exp = mybir.ActivationFunctionType.Exp
    Copy = mybir.ActivationFunctionType.Copy
    Ident = mybir.ActivationFunctionType.Identity
    Sqrt = mybir.ActivationFunctionType.Sqrt
    AX = mybir.AxisListType.X
    DR = mybir.MatmulPerfMode.DoubleRow
    MUL = mybir.AluOpType.mult
    ADD = mybir.AluOpType.add

    const = tc.alloc_tile_pool(name="const", bufs=1)
    attn = tc.alloc_tile_pool(name="attn", bufs=1)
    resid_pool = tc.alloc_tile_pool(name="resid", bufs=1)

    # ---- constants -------------------------------------------------------
    ident_bf = const.tile([128, 128], BF16)
    make_identity(nc, ident_bf)
    gmap_sb = const.tile([128, 8], F32)
    nc.sync.dma_start(out=gmap_sb, in_=gmap)
    gmapT_sb = const.tile([8, 128], F32)
    nc.sync.dma_start(out=gmapT_sb, in_=gmapT)
    # per-channel vectors as [128, CT] tiles: [p, i] = vec[i*128 + p]
    chan = {}
    for name in ("bq8", "bk8", "gamma", "beta"):
        t = const.tile([128, CT], F32, name=f"ch_{name}")
        nc.gpsimd.dma_start(out=t, in_=bs[name].rearrange("(i p) -> p i", p=128))
        chan[name] = t

    def bcast_rows(ap):
        import concourse.bass as bass

        return bass.AP(tensor=ap.tensor, offset=ap.offset, ap=[[0, 128], *ap.ap])

    bo2_bc = const.tile([128, C], F32)
    nc.gpsimd.dma_start(out=bo2_bc, in_=bcast_rows(bs["bo2"]))
    eps_t = const.tile([8, 1], F32)
    nc.vector.memset(eps_t, EPS)
    scl_t = const.tile([128, 1], F32)
    nc.vector.memset(scl_t, SCALE / (W_SCALE * W_SCALE))
    eb_t = const.tile([128, 1], F32)
    nc.vector.memset(eb_t, EXP_BIAS)
    ones_dr = const.tile([128, 2, 16], FP8)
    nc.vector.memset(ones_dr, 1.0)

    # weights (fp8, pre-scaled x8 on host); loaded after the x stream starts
    w_sb = {
        name: const.tile([128, CT, C], FP8, name=f"{name}_sb")
        for name in ("wq", "wk", "wvo")
    }

    sums_blk = const.tile([128, CT, JG], F32)
    sq_blk = const.tile([128, CT, JG], F32)
    stats = const.tile([128, 8], F32)       # cols 0..3 sum_i, 4..7 sumsq_i
    scale_sb = const.tile([128, CT], F32)
    bias_sb = const.tile([128, CT], F32)

    # persistent attention operands (fp8)
    kT = attn.tile([128, CT, N], FP8)
    qT = attn.tile([128, CT, NQ], FP8)
    v_sb = attn.tile([128, NT, C], FP8)

    # residual rows (this core's 2048 query rows), bf16, kept to the end
    resid = resid_pool.tile([128, 16, 512], BF16)

    xt_pool = tc.alloc_tile_pool(name="xt_pool", bufs=1)
    xT = xt_pool.tile([128, CT, N], BF16)    # [p, i, n] = x[n, i*128+p]

    # ---- phase 1: load + transpose + groupnorm stats ---------------------
    with (
        tc.tile_pool(name="ph1a", bufs=1) as ph1a,
        tc.tile_pool(name="ph1ps", bufs=1, space="PSUM") as ph1ps,
    ):
        for jg in range(JG):
            stgs = []
            for q in range(8):
                j = jg * 8 + q
                dma_eng = nc.sync if j % 2 == 0 else nc.scalar
                if j < 16:
                    stg = resid[:, j, :]
                    dma_eng.dma_start(out=stg, in_=x[j * 128 : (j + 1) * 128, :])
                else:
                    stg = ph1a.tile(
                        [128, 512], BF16, tag=f"xstage{q % 4}", bufs=2, name=f"stg{j}"
                    )
                    dma_eng.dma_start(out=stg, in_=x[j * 128 : (j + 1) * 128, :])
                stgs.append(stg)
            gsl = slice(jg * 1024, (jg + 1) * 1024)
            for i in range(CT):
                tp = ph1ps.tile([128, 8, 128], BF16, tag="tp", bufs=4, name=f"tp{jg}_{i}")
                for q in range(8):
                    nc.tensor.transpose(
                        tp[:, q, :],
                        stgs[q][:, i * 128 : (i + 1) * 128],
                        ident_bf,
                    )
                # PSUM -> SBUF bf16 evac, with per-channel sum accumulation
                nc.scalar.activation(
                    out=xT[:, i, gsl], in_=tp, func=Copy,
                    accum_out=sums_blk[:, i, jg : jg + 1],
                )
                sqs = ph1a.tile(
                    [128, 1024], BF16, tag="sqs", bufs=2, name=f"sq{jg}_{i}"
                )
                nc.vector.tensor_mul(out=sqs, in0=xT[:, i, gsl], in1=xT[:, i, gsl])
                nc.vector.tensor_scalar(
                    out=sqs, in0=sqs, scalar1=0.0, scalar2=0.0,
                    op0=ADD, op1=ADD,
                    accum_out=sq_blk[:, i, jg : jg + 1],
                )

        # weights can stream in behind the x stages
        for name in ("wk", "wq", "wvo"):
            for ci in range(CT):
                nc.sync.dma_start(
                    out=w_sb[name][:, ci, :],
                    in_=ws[name][ci * 128 : (ci + 1) * 128, :],
                )

        nc.vector.reduce_sum(out=stats[:, 0:4], in_=sums_blk, axis=AX)
        nc.vector.reduce_sum(out=stats[:, 4:8], in_=sq_blk, axis=AX)

    ph23ps = tc.alloc_tile_pool(name="ph23ps", bufs=1, space="PSUM")
    # group stats: [8, 8] = gmap^T @ stats;  cols 0..3 gsum, 4..7 gsumsq
    gs_ps = ph23ps.tile([8, 8], F32, tag="gs", bufs=1)
    nc.tensor.matmul(gs_ps, lhsT=gmap_sb, rhs=stats, start=True, stop=True)
    gstats = const.tile([8, 8], F32)
    nc.vector.tensor_copy(out=gstats, in_=gs_ps)

    inv_n = 1.0 / (N * GS)
    me_t = const.tile([8, 2 * CT], F32)     # cols 0..3 mean, 4..7 E[x^2]
    nc.vector.tensor_scalar_mul(out=me_t, in0=gstats, scalar1=inv_n)
    var_t = const.tile([8, CT], F32)
    nc.vector.tensor_mul(out=var_t, in0=me_t[:, 0:4], in1=me_t[:, 0:4])
    nc.vector.tensor_sub(out=var_t, in0=me_t[:, 4:8], in1=var_t)
    rstd_t = const.tile([8, CT], F32)
    nc.scalar.activation(out=rstd_t, in_=var_t, func=Sqrt, bias=eps_t)
    nc.vector.reciprocal(out=rstd_t, in_=rstd_t)

    # broadcast per-group -> per-channel: bc_ps cols 2i=mean_i, 2i+1=rstd_i
    bc_ps = ph23ps.tile([128, 2 * CT], F32, tag="gs", bufs=1, name="bc")
    for i in range(CT):
        nc.tensor.matmul(
            bc_ps[:, 2 * i : 2 * i + 1], lhsT=gmapT_sb,
            rhs=me_t[:, i : i + 1], start=True, stop=True,
        )
        nc.tensor.matmul(
            bc_ps[:, 2 * i + 1 : 2 * i + 2], lhsT=gmapT_sb,
            rhs=rstd_t[:, i : i + 1], start=True, stop=True,
        )
    tmp4 = const.tile([128, CT], F32)
    nc.vector.tensor_mul(out=scale_sb, in0=chan["gamma"], in1=bc_ps[:, 1:8:2])
    nc.vector.tensor_mul(out=tmp4, in0=bc_ps[:, 0:8:2], in1=scale_sb)
    nc.vector.tensor_sub(out=bias_sb, in0=chan["beta"], in1=tmp4)

    # ---- phase 2: normalize (fp8) + K/Q/V --------------------------------
    # PSUM evacuations are legal only on Act/DVE; alternate between them.
    # PSUM_SPLIT: emit PSUM reads per 2KB bank (in case HW can't cross banks).
    PSUM_SPLIT = False
    evac_rr = [0]

    def psum_evac(out, in_, bias=None):
        if PSUM_SPLIT:
            # fp32 [128, 2, 512] psum tile = 2 banks; split reads per bank
            if len(out.shape) == 3:
                pieces = [(out[:, 0, :], in_[:, 0, :]), (out[:, 1, :], in_[:, 1, :])]
            else:
                pieces = [(out[:, 0:512], in_[:, 0, :]), (out[:, 512:1024], in_[:, 1, :])]
        else:
            pieces = [(out, in_)]
        evac_rr[0] ^= 1
        for o_, i_ in pieces:
            if evac_rr[0]:
                nc.scalar.activation(
                    out=o_, in_=i_, func=(Ident if bias is not None else Copy),
                    **({"bias": bias} if bias is not None else {}),
                )
            elif bias is not None:
                nc.vector.tensor_scalar_add(out=o_, in0=i_, scalar1=bias)
            else:
                nc.vector.tensor_copy(out=o_, in_=i_)

    tt_pool = tc.alloc_tile_pool(name="tt_pool", bufs=1)
    tT = tt_pool.tile([128, CT, N], FP8)
    for cp in range(CP):
        sl = slice(cp * 1024, (cp + 1) * 1024)
        for i in range(CT):
            # normalize (SBUF->SBUF) on gpsimd: keeps Act/DVE free for evacs
            GPSIMD_COMPUTE = True
            eng = nc.gpsimd if GPSIMD_COMPUTE else nc.vector
            eng.tensor_scalar(
                out=tT[:, i, sl], in0=xT[:, i, sl],
                scalar1=scale_sb[:, i : i + 1], scalar2=bias_sb[:, i : i + 1],
                op0=MUL, op1=ADD,
            )
        # K^T chunk-pair: DR fp8, Act evac (+8*bk)
        for i in range(CT):
            kps = ph23ps.tile([128, 2, 512], F32, tag="mm", bufs=3, name=f"k{cp}_{i}")
            for h in range(2):
                hsl = slice(cp * 1024 + h * 512, cp * 1024 + (h + 1) * 512)
                for a in range(2):
                    nc.tensor.matmul(
                        kps[:, h, :],
                        lhsT=w_sb["wk"][:, 2 * a : 2 * a + 2, i * 128 : (i + 1) * 128],
                        rhs=tT[:, 2 * a : 2 * a + 2, hsl],
                        start=(a == 0), stop=(a == 1),
                        perf_mode=DR,
                    )
            psum_evac(kT[:, i, sl], kps, bias=chan["bk8"][:, i : i + 1])
        # Q^T chunk-pair (tokens [0, NQ) are this core's queries): DVE evac
        if cp < NQ // 1024:
            for i in range(CT):
                qps = ph23ps.tile([128, 2, 512], F32, tag="mm", bufs=3, name=f"q{cp}_{i}")
                for h in range(2):
                    hsl = slice(cp * 1024 + h * 512, cp * 1024 + (h + 1) * 512)
                    for a in range(2):
                        nc.tensor.matmul(
                            qps[:, h, :],
                            lhsT=w_sb["wq"][:, 2 * a : 2 * a + 2, i * 128 : (i + 1) * 128],
                            rhs=tT[:, 2 * a : 2 * a + 2, hsl],
                            start=(a == 0), stop=(a == 1),
                            perf_mode=DR,
                        )
                psum_evac(qT[:, i, sl], qps, bias=chan["bq8"][:, i : i + 1])
        # v2 m-tiles of this chunk-pair (Wvo fused; no bias): Pool evac
        for mp in range(4):
            m0 = cp * 8 + 2 * mp
            vps = ph23ps.tile([128, 2, 512], F32, tag="mm", bufs=3, name=f"v{cp}_{mp}")
            for h in range(2):
                m = m0 + h
                for a in range(2):
                    nc.tensor.matmul(
                        vps[:, h, :],
                        lhsT=tT[:, 2 * a : 2 * a + 2, m * 128 : (m + 1) * 128],
                        rhs=w_sb["wvo"][:, 2 * a : 2 * a + 2, :],
                        start=(a == 0), stop=(a == 1),
                        perf_mode=DR,
                    )
            psum_evac(v_sb[:, m0 : m0 + 2, :], vps)

    tt_pool.release()
    ph23ps.release()
    xt_pool.release()

    # ---- phase 3: attention ---------------------------------------------
    with (
        tc.tile_pool(name="ph3", bufs=1) as ph3,
        tc.tile_pool(name="ph3ps", bufs=1, space="PSUM") as ph3ps,
    ):
        def emit_O_half(st, ho):
            """O'^T channel half [2*ho*128, (2*ho+2)*128) for a finished q-chunk."""
            qc, p_all, oT, _rd = st
            o_ps = ph3ps.tile(
                [128, 2, 512], F32, tag="o", bufs=1, name=f"o{qc}_{ho}"
            )
            for b in range(NPAIR):
                for i2 in range(2):
                    i = 2 * ho + i2
                    nc.tensor.matmul(
                        o_ps[:, i2, :],
                        lhsT=v_sb[:, 2 * b : 2 * b + 2, i * 128 : (i + 1) * 128],
                        rhs=p_all[:, 2 * b : 2 * b + 2, :],
                        start=(b == 0), stop=(b == NPAIR - 1),
                        skip_group_check=True,
                        perf_mode=DR,
                    )
            nc.vector.tensor_copy(out=oT[:, 2 * ho : 2 * ho + 2, :], in_=o_ps)

        def emit_fin(st, s):
            """transpose + scale + residual + store for one 128-row out tile."""
            qc, _p_all, oT, rd = st
            ftr = ph3ps.tile([128, 512], BF16, tag="ftr", bufs=1, name=f"ftr{qc}_{s}")
            for i in range(CT):
                nc.tensor.transpose(
                    ftr[:, i * 128 : (i + 1) * 128],
                    oT[:, i, s * 128 : (s + 1) * 128],
                    ident_bf,
                )
            fin = ph3.tile([128, C], F32, tag="fin", bufs=3, name=f"fin{qc}_{s}")
            nc.vector.scalar_tensor_tensor(
                out=fin, in0=ftr, scalar=rd[:, s : s + 1],
                in1=resid[:, qc * 4 + s, :], op0=MUL, op1=ADD,
            )
            nc.vector.tensor_add(out=fin, in0=fin, in1=bo2_bc)
            row0 = qc * 512 + s * 128
            nc.sync.dma_start(out=out[row0 : row0 + 128, :], in_=fin)

        prev = None       # (qc, p_all, oT, rd) of the previous q-chunk
        for qc in range(QC):
            qsl = slice(qc * 512, (qc + 1) * 512)
            p_all = ph3.tile([128, NT, 512], FP8, tag="p", bufs=2, name=f"p{qc}")
            oT = ph3.tile([128, CT, 512], BF16, tag="oT", bufs=2, name=f"oT{qc}")
            den_ps = ph3ps.tile([1, 512], F32, tag="den", bufs=1, name=f"dps{qc}")
            for b in range(NPAIR):
                s_big = ph3ps.tile(
                    [128, 2, 512], F32, tag="s", bufs=2, name=f"s{qc}_{b}"
                )
                for h in range(2):
                    m = 2 * b + h
                    for a in range(2):
                        nc.tensor.matmul(
                            s_big[:, h, :],
                            lhsT=kT[:, 2 * a : 2 * a + 2, m * 128 : (m + 1) * 128],
                            rhs=qT[:, 2 * a : 2 * a + 2, qsl],
                            start=(a == 0), stop=(a == 1),
                            perf_mode=DR,
                        )
                nc.scalar.activation(
                    out=p_all[:, 2 * b : 2 * b + 2, :], in_=s_big, func=Exp,
                    scale=scl_t, bias=eb_t,
                )
                nc.tensor.matmul(
                    den_ps, lhsT=ones_dr[:, :, 0:1],
                    rhs=p_all[:, 2 * b : 2 * b + 2, :],
                    start=(b == 0), stop=(b == NPAIR - 1),
                    skip_group_check=True,
                    perf_mode=DR,
                )
                if prev is not None:
                    if b == 2:
                        emit_O_half(prev, 0)
                    elif b == 6:
                        emit_O_half(prev, 1)
                    elif b == 10:
                        emit_fin(prev, 0)
                        emit_fin(prev, 1)
                    elif b == 13:
                        emit_fin(prev, 2)
                        emit_fin(prev, 3)

            # denominator (x8 for the fp8 weight scale): DMA-transpose to rd
            den_sb = ph3.tile([1, 512], F32, tag="den_sb", bufs=2, name=f"dsb{qc}")
            nc.vector.tensor_scalar_mul(out=den_sb, in0=den_ps, scalar1=W_SCALE)
            nc.sync.dma_start(out=den_dram[qc : qc + 1, :], in_=den_sb)
            rd = ph3.tile([128, 4], F32, tag="rd", bufs=2, name=f"rd{qc}")
            nc.gpsimd.dma_start(
                out=rd, in_=den_dram[qc, :].rearrange("(s p) -> p s", p=128)
            )
            nc.vector.reciprocal(out=rd, in_=rd)
            prev = (qc, p_all, oT, rd)

        emit_O_half(prev, 0)
        emit_O_half(prev, 1)
        for s in range(4):
            emit_fin(prev, s)

    resid_pool.release()
    attn.release()
    const.release()


_prog_cache = None


def get_program():
    global _prog_cache
    if _prog_cache is None:
        _prog_cache = build_program()
    return _prog_cache


def make_gmaps():
    gmap = np.zeros((128, 8), np.float32)
    gmap[np.arange(128), np.arange(128) // GS] = 1.0
    return gmap, np.ascontiguousarray(gmap.T)


def make_in_maps(inputs):
    x = np.asarray(inputs["x"], np.float32)          # [B, H, W, C]
    gmap, gmapT = make_gmaps()
    f32 = np.float32
    Wq = np.asarray(inputs["Wq"], f32)
    Wk = np.asarray(inputs["Wk"], f32)
    Wv = np.asarray(inputs["Wv"], f32)
    Wo = np.asarray(inputs["Wo"], f32)
    Wvo = (Wv @ Wo).astype(f32)
    bo2 = (np.asarray(inputs["bo"], f32)
           + np.asarray(inputs["bv"], f32) @ Wo).astype(f32)

    def fp8(a):
        return np.ascontiguousarray(np.asarray(a, dtype=ml_dtypes.float8_e4m3))

    common = {
        "wq": fp8(W_SCALE * Wq),
        "wk": fp8(W_SCALE * Wk),
        "wvo": fp8(W_SCALE * Wvo),
        "bq8": np.ascontiguousarray(W_SCALE * np.asarray(inputs["bq"], f32)),
        "bk8": np.ascontiguousarray(W_SCALE * np.asarray(inputs["bk"], f32)),
        "bo2": np.ascontiguousarray(bo2),
        "gamma": np.ascontiguousarray(np.asarray(inputs["gn_gamma"], f32)),
        "beta": np.ascontiguousarray(np.asarray(inputs["gn_beta"], f32)),
        "gmap": gmap,
        "gmapT": gmapT,
    }
    in_maps = []
    for core in range(N_CORES):
        b, h = divmod(core, 2)
        xs = x[b].reshape(N, C)
        if h:
            xs = np.roll(xs, -NQ, axis=0)
        in_maps.append(
            {"x": np.ascontiguousarray(xs.astype(ml_dtypes.bfloat16)), **common}
        )
    return in_maps


def assemble(results):
    full = np.empty((B, N, C), np.float32)
    for core in range(N_CORES):
        b, h = divmod(core, 2)
        full[b, h * NQ : (h + 1) * NQ] = results[core]["out"]
    return full.reshape(B, HH, WW, C)


def kernel(**inputs) -> np.ndarray:
    in_maps = make_in_maps(inputs)
    nc = get_program()
    res = bass_utils.run_bass_kernel_spmd(nc, in_maps, core_ids=list(range(N_CORES)))
    return assemble(res.results)


# revision 29
# speedup vs baseline: 1.4531x; 1.0555x over previous
"""AttentionBlock (GroupNorm + single-head self-attention + residual) on 8 TRN2 cores.

Sharding: data-parallel over batch (B=4) x query-halves (2 per sample) = 8 cores.
Each core gets one full (row-rotated) sample [4096, 512]; the rotation puts that
core's 2048 query rows at rows [0, 2048) so all 8 cores run one identical SPMD
program. Softmax/attention are invariant to key-row permutation, so rotating
keys/values together with the sample is exact.

Host prep: x cast to bf16; weights cast to fp8 scaled by 8 (avoids fp8
subnormals); Wo folded into Wv (Wvo = Wv @ Wo) which eliminates the output
projection matmul; bv folded into bo2 = bo + bv @ Wo.

Per-core pipeline (fp8 DoubleRow for every large matmul):
  ph1: x [4096,512] bf16 --PE-transpose--> xT bf16; Act evacs PSUM->SBUF with
       channel-sum accum; DVE squares with accum -> groupnorm stats.
  ph2: tT = fp8(scale*xT + bias) (DVE);  K^T/Q^T = W8-blocks^T @ tT (DR fp8,
       Act/DVE evac + 8*bias);  v2 = tT-blocks^T @ W8vo (DR fp8, Pool evac).
  ph3: S^T[m,q] pairs (DR fp8) -> exp (Act, merged [128,1024]) -> P (fp8 SBUF);
       den[q] = ones^T P (DR);  O'^T[c,q] = v2^T P (DR, deferred one q-chunk);
       PE-transpose O' -> [q,c];  out = O'*(1/(8 den)) + resid + bo2 (Pool).
"""

import math

import numpy as np
import ml_dtypes

import concourse.bacc as bacc
import concourse.mybir as mybir
import concourse.tile as tile
from concourse import bass_utils
from concourse.masks import make_identity

B, HH, WW, C = 4, 64, 64, 512
N = HH * WW          # 4096 tokens per sample
NQ = N // 2          # 2048 queries per core
G = 32               # groupnorm groups
GS = C // G          # 16 channels per group
EPS = 1e-6
SCALE = 1.0 / math.sqrt(C)
N_CORES = 8
F32 = mybir.dt.float32
BF16 = mybir.dt.bfloat16
FP8 = mybir.dt.float8e4
W_SCALE = 8.0            # weights stored as fp8(8*W)
EXP_BIAS = -2.0          # exp(scale*S + bias): keeps fp8 p in [~1e-3, 320]

CT = C // 128        # 4 channel tiles
NT = N // 128        # 32 token tiles
JG = N // 1024       # 4 1024-token groups (phase 1)
CP = N // 1024       # 4 1024-token chunk-pairs (phase 2)
QC = NQ // 512       # 4 query chunks per core
NPAIR = NT // 2      # 16 m-tile pairs per q-chunk


def build_program():
    nc = bacc.Bacc("TRN2", target_bir_lowering=False, debug=False)

    x = nc.dram_tensor("x", [N, C], BF16, kind="ExternalInput").ap()
    ws = {
        w: nc.dram_tensor(w, [C, C], FP8, kind="ExternalInput").ap()
        for w in ("wq", "wk", "wvo")
    }
    bs = {
        b: nc.dram_tensor(b, [C], F32, kind="ExternalInput").ap()
        for b in ("bq8", "bk8", "bo2", "gamma", "beta")
    }
    gmap = nc.dram_tensor("gmap", [128, 8], F32, kind="ExternalInput").ap()
    gmapT = nc.dram_tensor("gmapT", [8, 128], F32, kind="ExternalInput").ap()
    out = nc.dram_tensor("out", [NQ, C], F32, kind="ExternalOutput").ap()
    den_dram = nc.dram_tensor("den_scratch", [QC, 512], F32, kind="Internal").ap()

    with tile.TileContext(nc) as tc:
        build_body(tc, x, ws, bs, gmap, gmapT, out, den_dram)
    nc.compile()
    return nc


def build_body(tc, x, ws, bs, gmap, gmapT, out, den_dram):
    nc = tc.nc
    Exp = mybir.ActivationFunctionType.Exp
    Copy = mybir.ActivationFunctionType.Copy
    Ident = mybir.ActivationFunctionType.Identity
    Sqrt = mybir.ActivationFunctionType.Sqrt
    AX = mybir.AxisListType.X
    DR = mybir.MatmulPerfMode.DoubleRow
    MUL = mybir.AluOpType.mult
    ADD = mybir.AluOpType.add

    const = tc.alloc_tile_pool(name="const", bufs=1)
    attn = tc.alloc_tile_pool(name="attn", bufs=1)
    resid_pool = tc.alloc_tile_pool(name="resid", bufs=1)

    # ---- constants -------------------------------------------------------
    ident_bf = const.tile([128, 128], BF16)
    make_identity(nc, ident_bf)
    gmap_sb = const.tile([128, 8], F32)
    nc.sync.dma_start(out=gmap_sb, in_=gmap)
    gmapT_sb = const.tile([8, 128], F32)
    nc.sync.dma_start(out=gmapT_sb, in_=gmapT)
    # per-channel vectors as [128, CT] tiles: [p, i] = vec[i*128 + p]
    chan = {}
    for name in ("bq8", "bk8", "gamma", "beta"):
        t = const.tile([128, CT], F32, name=f"ch_{name}")
        nc.gpsimd.dma_start(out=t, in_=bs[name].rearrange("(i p) -> p i", p=128))
        chan[name] = t

    def bcast_rows(ap):
        import concourse.bass as bass

        return bass.AP(tensor=ap.tensor, offset=ap.offset, ap=[[0, 128], *ap.ap])

    bo2_bc = const.tile([128, C], F32)
    nc.gpsimd.dma_start(out=bo2_bc, in_=bcast_rows(bs["bo2"]))
    eps_t = const.tile([8, 1], F32)
    nc.vector.memset(eps_t, EPS)
    scl_t = const.tile([128, 1], F32)
    nc.vector.memset(scl_t, SCALE / (W_SCALE * W_SCALE))
    eb_t = const.tile([128, 1], F32)
    nc.vector.memset(eb_t, EXP_BIAS)
    ones_dr = const.tile([128, 2, 16], FP8)
    nc.vector.memset(ones_dr, 1.0)

    # weights (fp8, pre-scaled x8 on host); loaded after the x stream starts
    w_sb = {
        name: const.tile([128, CT, C], FP8, name=f"{name}_sb")
        for name in ("wq", "wk", "wvo")
    }

    sums_blk = const.tile([128, CT, JG], F32)
    sq_blk = const.tile([128, CT, JG], F32)
    stats = const.tile([128, 8], F32)       # cols 0..3 sum_i, 4..7 sumsq_i
    scale_sb = const.tile([128, CT], F32)
    bias_sb = const.tile([128, CT], F32)

    # persistent attention operands (fp8)
    kT = attn.tile([128, CT, N], FP8)
    qT = attn.tile([128, CT, NQ], FP8)
    v_sb = attn.tile([128, NT, C], FP8)

    # residual rows (this core's 2048 query rows), bf16, kept to the end
    resid = resid_pool.tile([128, 16, 512], BF16)

    xt_pool = tc.alloc_tile_pool(name="xt_pool", bufs=1)
    xT = xt_pool.tile([128, CT, N], BF16)    # [p, i, n] = x[n, i*128+p]

    # ---- phase 1: load + transpose + groupnorm stats ---------------------
    with (
        tc.tile_pool(name="ph1a", bufs=1) as ph1a,
        tc.tile_pool(name="ph1ps", bufs=1, space="PSUM") as ph1ps,
    ):
        for jg in range(JG):
            dma_eng = nc.sync if jg % 2 == 0 else nc.scalar
            xsl = x[jg * 1024 : (jg + 1) * 1024, :].rearrange(
                "(k p) c -> p k c", p=128
            )
            if jg < 2:
                stg8 = resid[:, jg * 8 : (jg + 1) * 8, :]
            else:
                stg8 = ph1a.tile(
                    [128, 8, 512], BF16, tag="xstage", bufs=2, name=f"stg{jg}"
                )
            dma_eng.dma_start(out=stg8, in_=xsl)
            stgs = [stg8[:, q, :] for q in range(8)]
            gsl = slice(jg * 1024, (jg + 1) * 1024)
            for i in range(CT):
                tp = ph1ps.tile([128, 8, 128], BF16, tag="tp", bufs=4, name=f"tp{jg}_{i}")
                for q in range(8):
                    nc.tensor.transpose(
                        tp[:, q, :],
                        stgs[q][:, i * 128 : (i + 1) * 128],
                        ident_bf,
                    )
                # PSUM -> SBUF bf16 evac, with per-channel sum accumulation
                nc.scalar.activation(
                    out=xT[:, i, gsl], in_=tp, func=Copy,
                    accum_out=sums_blk[:, i, jg : jg + 1],
                )
                sqs = ph1a.tile(
                    [128, 1024], BF16, tag="sqs", bufs=2, name=f"sq{jg}_{i}"
                )
                nc.vector.tensor_mul(out=sqs, in0=xT[:, i, gsl], in1=xT[:, i, gsl])
                nc.vector.tensor_scalar(
                    out=sqs, in0=sqs, scalar1=0.0, scalar2=0.0,
                    op0=ADD, op1=ADD,
                    accum_out=sq_blk[:, i, jg : jg + 1],
                )

        # weights can stream in behind the x stages (one DMA per matrix)
        for qi, name in enumerate(("wk", "wq", "wvo")):
            (nc.sync if qi % 2 == 0 else nc.scalar).dma_start(
                out=w_sb[name],
                in_=ws[name].rearrange("(i p) c -> p i c", p=128),
            )

        nc.vector.reduce_sum(out=stats[:, 0:4], in_=sums_blk, axis=AX)
        nc.vector.reduce_sum(out=stats[:, 4:8], in_=sq_blk, axis=AX)

    ph23ps = tc.alloc_tile_pool(name="ph23ps", bufs=1, space="PSUM")
    # group stats: [8, 8] = gmap^T @ stats;  cols 0..3 gsum, 4..7 gsumsq
    gs_ps = ph23ps.tile([8, 8], F32, tag="gs", bufs=1)
    nc.tensor.matmul(gs_ps, lhsT=gmap_sb, rhs=stats, start=True, stop=True)
    gstats = const.tile([8, 8], F32)
    nc.vector.tensor_copy(out=gstats, in_=gs_ps)

    inv_n = 1.0 / (N * GS)
    me_t = const.tile([8, 2 * CT], F32)     # cols 0..3 mean, 4..7 E[x^2]
    nc.vector.tensor_scalar_mul(out=me_t, in0=gstats, scalar1=inv_n)
    var_t = const.tile([8, CT], F32)
    nc.vector.tensor_mul(out=var_t, in0=me_t[:, 0:4], in1=me_t[:, 0:4])
    nc.vector.tensor_sub(out=var_t, in0=me_t[:, 4:8], in1=var_t)
    rstd_t = const.tile([8, CT], F32)
    nc.scalar.activation(out=rstd_t, in_=var_t, func=Sqrt, bias=eps_t)
    nc.vector.reciprocal(out=rstd_t, in_=rstd_t)

    # broadcast per-group -> per-channel: bc_ps cols 2i=mean_i, 2i+1=rstd_i
    bc_ps = ph23ps.tile([128, 2 * CT], F32, tag="gs", bufs=1, name="bc")
    for i in range(CT):
        nc.tensor.matmul(
            bc_ps[:, 2 * i : 2 * i + 1], lhsT=gmapT_sb,
            rhs=me_t[:, i : i + 1], start=True, stop=True,
        )
        nc.tensor.matmul(
            bc_ps[:, 2 * i + 1 : 2 * i + 2], lhsT=gmapT_sb,
            rhs=rstd_t[:, i : i + 1], start=True, stop=True,
        )
    tmp4 = const.tile([128, CT], F32)
    nc.vector.tensor_mul(out=scale_sb, in0=chan["gamma"], in1=bc_ps[:, 1:8:2])
    nc.vector.tensor_mul(out=tmp4, in0=bc_ps[:, 0:8:2], in1=scale_sb)
    nc.vector.tensor_sub(out=bias_sb, in0=chan["beta"], in1=tmp4)

    # ---- phase 2: normalize (fp8) + K/Q/V --------------------------------
    # PSUM evacuations are legal only on Act/DVE; alternate between them.
    # PSUM_SPLIT: emit PSUM reads per 2KB bank (in case HW can't cross banks).
    PSUM_SPLIT = False
    evac_rr = [0]

    def psum_evac(out, in_, bias=None):
        if PSUM_SPLIT:
            # fp32 [128, 2, 512] psum tile = 2 banks; split reads per bank
            if len(out.shape) == 3:
                pieces = [(out[:, 0, :], in_[:, 0, :]), (out[:, 1, :], in_[:, 1, :])]
            else:
                pieces = [(out[:, 0:512], in_[:, 0, :]), (out[:, 512:1024], in_[:, 1, :])]
        else:
            pieces = [(out, in_)]
        evac_rr[0] ^= 1
        for o_, i_ in pieces:
            if evac_rr[0]:
                nc.scalar.activation(
                    out=o_, in_=i_, func=(Ident if bias is not None else Copy),
                    **({"bias": bias} if bias is not None else {}),
                )
            elif bias is not None:
                nc.vector.tensor_scalar_add(out=o_, in0=i_, scalar1=bias)
            else:
                nc.vector.tensor_copy(out=o_, in_=i_)

    tt_pool = tc.alloc_tile_pool(name="tt_pool", bufs=1)
    tT = tt_pool.tile([128, CT, N], FP8)
    for cp in range(CP):
        sl = slice(cp * 1024, (cp + 1) * 1024)
        for i in range(CT):
            # normalize (SBUF->SBUF): DVE for the i-blocks the K matmul needs
            # first, gpsimd for the rest (keeps Act/DVE free for evacs)
            eng = nc.vector if i < 2 else nc.gpsimd
            eng.tensor_scalar(
                out=tT[:, i, sl], in0=xT[:, i, sl],
                scalar1=scale_sb[:, i : i + 1], scalar2=bias_sb[:, i : i + 1],
                op0=MUL, op1=ADD,
            )
        # K^T chunk-pair: DR fp8, Act evac (+8*bk)
        for i in range(CT):
            kps = ph23ps.tile([128, 2, 512], F32, tag="mm", bufs=3, name=f"k{cp}_{i}")
            for h in range(2):
                hsl = slice(cp * 1024 + h * 512, cp * 1024 + (h + 1) * 512)
                for a in range(2):
                    nc.tensor.matmul(
                        kps[:, h, :],
                        lhsT=w_sb["wk"][:, 2 * a : 2 * a + 2, i * 128 : (i + 1) * 128],
                        rhs=tT[:, 2 * a : 2 * a + 2, hsl],
                        start=(a == 0), stop=(a == 1),
                        perf_mode=DR,
                    )
            psum_evac(kT[:, i, sl], kps, bias=chan["bk8"][:, i : i + 1])
        # Q^T chunk-pair (tokens [0, NQ) are this core's queries): DVE evac
        if cp < NQ // 1024:
            for i in range(CT):
                qps = ph23ps.tile([128, 2, 512], F32, tag="mm", bufs=3, name=f"q{cp}_{i}")
                for h in range(2):
                    hsl = slice(cp * 1024 + h * 512, cp * 1024 + (h + 1) * 512)
                    for a in range(2):
                        nc.tensor.matmul(
                            qps[:, h, :],
                            lhsT=w_sb["wq"][:, 2 * a : 2 * a + 2, i * 128 : (i + 1) * 128],
                            rhs=tT[:, 2 * a : 2 * a + 2, hsl],
                            start=(a == 0), stop=(a == 1),
                            perf_mode=DR,
                        )
                psum_evac(qT[:, i, sl], qps, bias=chan["bq8"][:, i : i + 1])
        # v2 m-tiles of this chunk-pair (Wvo fused; no bias): Pool evac
        for mp in range(4):
            m0 = cp * 8 + 2 * mp
            vps = ph23ps.tile([128, 2, 512], F32, tag="mm", bufs=3, name=f"v{cp}_{mp}")
            for h in range(2):
                m = m0 + h
                for a in range(2):
                    nc.tensor.matmul(
                        vps[:, h, :],
                        lhsT=tT[:, 2 * a : 2 * a + 2, m * 128 : (m + 1) * 128],
                        rhs=w_sb["wvo"][:, 2 * a : 2 * a + 2, :],
                        start=(a == 0), stop=(a == 1),
                        perf_mode=DR,
                    )
            psum_evac(v_sb[:, m0 : m0 + 2, :], vps)

    tt_pool.release()
    ph23ps.release()
    xt_pool.release()

    # ---- phase 3: attention ---------------------------------------------
    with (
        tc.tile_pool(name="ph3", bufs=1) as ph3,
        tc.tile_pool(name="ph3ps", bufs=1, space="PSUM") as ph3ps,
    ):
        def emit_O_half(st, ho):
            """O'^T channel half [2*ho*128, (2*ho+2)*128) for a finished q-chunk."""
            qc, p_all, oT, _rd = st
            o_ps = ph3ps.tile(
                [128, 2, 512], F32, tag="o", bufs=1, name=f"o{qc}_{ho}"
            )
            for b in range(NPAIR):
                for i2 in range(2):
                    i = 2 * ho + i2
                    nc.tensor.matmul(
                        o_ps[:, i2, :],
                        lhsT=v_sb[:, 2 * b : 2 * b + 2, i * 128 : (i + 1) * 128],
                        rhs=p_all[:, 2 * b : 2 * b + 2, :],
                        start=(b == 0), stop=(b == NPAIR - 1),
                        skip_group_check=True,
                        perf_mode=DR,
                    )
            nc.vector.tensor_copy(out=oT[:, 2 * ho : 2 * ho + 2, :], in_=o_ps)

        def emit_fin(st, s):
            """transpose + scale + residual + store for one 128-row out tile."""
            qc, _p_all, oT, rd = st
            ftr = ph3ps.tile([128, 512], BF16, tag="ftr", bufs=1, name=f"ftr{qc}_{s}")
            for i in range(CT):
                nc.tensor.transpose(
                    ftr[:, i * 128 : (i + 1) * 128],
                    oT[:, i, s * 128 : (s + 1) * 128],
                    ident_bf,
                )
            fin = ph3.tile([128, C], F32, tag="fin", bufs=3, name=f"fin{qc}_{s}")
            nc.vector.scalar_tensor_tensor(
                out=fin, in0=ftr, scalar=rd[:, s : s + 1],
                in1=resid[:, qc * 4 + s, :], op0=MUL, op1=ADD,
            )
            nc.vector.tensor_add(out=fin, in0=fin, in1=bo2_bc)
            row0 = qc * 512 + s * 128
            nc.sync.dma_start(out=out[row0 : row0 + 128, :], in_=fin)

        prev = None       # (qc, p_all, oT, rd) of the previous q-chunk
        for qc in range(QC):
            qsl = slice(qc * 512, (qc + 1) * 512)
            p_all = ph3.tile([128, NT, 512], FP8, tag="p", bufs=2, name=f"p{qc}")
            oT = ph3.tile([128, CT, 512], BF16, tag="oT", bufs=2, name=f"oT{qc}")
            den_ps = ph3ps.tile([1, 512], F32, tag="den", bufs=1, name=f"dps{qc}")
            for b in range(NPAIR):
                s_big = ph3ps.tile(
                    [128, 2, 512], F32, tag="s", bufs=2, name=f"s{qc}_{b}"
                )
                for h in range(2):
                    m = 2 * b + h
                    for a in range(2):
                        nc.tensor.matmul(
                            s_big[:, h, :],
                            lhsT=kT[:, 2 * a : 2 * a + 2, m * 128 : (m + 1) * 128],
                            rhs=qT[:, 2 * a : 2 * a + 2, qsl],
                            start=(a == 0), stop=(a == 1),
                            perf_mode=DR,
                        )
                nc.scalar.activation(
                    out=p_all[:, 2 * b : 2 * b + 2, :], in_=s_big, func=Exp,
                    scale=scl_t, bias=eb_t,
                )
                nc.tensor.matmul(
                    den_ps, lhsT=ones_dr[:, :, 0:1],
                    rhs=p_all[:, 2 * b : 2 * b + 2, :],
                    start=(b == 0), stop=(b == NPAIR - 1),
                    skip_group_check=True,
                    perf_mode=DR,
                )
                if prev is not None:
                    if b == 2:
                        emit_O_half(prev, 0)
                    elif b == 6:
                        emit_O_half(prev, 1)
                    elif b == 10:
                        emit_fin(prev, 0)
                        emit_fin(prev, 1)
                    elif b == 13:
                        emit_fin(prev, 2)
                        emit_fin(prev, 3)

            # denominator (x8 for the fp8 weight scale): DMA-transpose to rd
            den_sb = ph3.tile([1, 512], F32, tag="den_sb", bufs=2, name=f"dsb{qc}")
            nc.vector.tensor_scalar_mul(out=den_sb, in0=den_ps, scalar1=W_SCALE)
            nc.sync.dma_start(out=den_dram[qc : qc + 1, :], in_=den_sb)
            rd = ph3.tile([128, 4], F32, tag="rd", bufs=2, name=f"rd{qc}")
            nc.gpsimd.dma_start(
                out=rd, in_=den_dram[qc, :].rearrange("(s p) -> p s", p=128)
            )
            nc.vector.reciprocal(out=rd, in_=rd)
            prev = (qc, p_all, oT, rd)

        emit_O_half(prev, 0)
        emit_O_half(prev, 1)
        for s in range(4):
            emit_fin(prev, s)

    resid_pool.release()
    attn.release()
    const.release()


_prog_cache = None


def get_program():
    global _prog_cache
    if _prog_cache is None:
        _prog_cache = build_program()
    return _prog_cache


def make_gmaps():
    gmap = np.zeros((128, 8), np.float32)
    gmap[np.arange(128), np.arange(128) // GS] = 1.0
    return gmap, np.ascontiguousarray(gmap.T)


def make_in_maps(inputs):
    x = np.asarray(inputs["x"], np.float32)          # [B, H, W, C]
    gmap, gmapT = make_gmaps()
    f32 = np.float32
    Wq = np.asarray(inputs["Wq"], f32)
    Wk = np.asarray(inputs["Wk"], f32)
    Wv = np.asarray(inputs["Wv"], f32)
    Wo = np.asarray(inputs["Wo"], f32)
    Wvo = (Wv @ Wo).astype(f32)
    bo2 = (np.asarray(inputs["bo"], f32)
           + np.asarray(inputs["bv"], f32) @ Wo).astype(f32)

    def fp8(a):
        return np.ascontiguousarray(np.asarray(a, dtype=ml_dtypes.float8_e4m3))

    common = {
        "wq": fp8(W_SCALE * Wq),
        "wk": fp8(W_SCALE * Wk),
        "wvo": fp8(W_SCALE * Wvo),
        "bq8": np.ascontiguousarray(W_SCALE * np.asarray(inputs["bq"], f32)),
        "bk8": np.ascontiguousarray(W_SCALE * np.asarray(inputs["bk"], f32)),
        "bo2": np.ascontiguousarray(bo2),
        "gamma": np.ascontiguousarray(np.asarray(inputs["gn_gamma"], f32)),
        "beta": np.ascontiguousarray(np.asarray(inputs["gn_beta"], f32)),
        "gmap": gmap,
        "gmapT": gmapT,
    }
    in_maps = []
    for core in range(N_CORES):
        b, h = divmod(core, 2)
        xs = x[b].reshape(N, C)
        if h:
            xs = np.roll(xs, -NQ, axis=0)
        in_maps.append(
            {"x": np.ascontiguousarray(xs.astype(ml_dtypes.bfloat16)), **common}
        )
    return in_maps


def assemble(results):
    full = np.empty((B, N, C), np.float32)
    for core in range(N_CORES):
        b, h = divmod(core, 2)
        full[b, h * NQ : (h + 1) * NQ] = results[core]["out"]
    return full.reshape(B, HH, WW, C)


def kernel(**inputs) -> np.ndarray:
    in_maps = make_in_maps(inputs)
    nc = get_program()
    res = bass_utils.run_bass_kernel_spmd(nc, in_maps, core_ids=list(range(N_CORES)))
    return assemble(res.results)
